# revision 5
# baseline (speedup 1.0000x reference)
"""Trainium2 Bass kernel for nn_AttentionBlock (B=32, C=512, H=W=64, GROUPS=32).

Sharding: data-parallel over batch — 4 samples per core on 8 NeuronCores,
no collectives. Host preprocesses weights (transpose, prescale q by
1/sqrt(L), round to float32r) and sends small indicator matrices used for
cross-partition GroupNorm reductions on the tensor engine.

Per-sample dataflow on one core (x viewed as (C=512, L=4096), channels on
partitions, 4 c-tiles of 128):
  1. GroupNorm stats: per-partition sum (DVE reduce) + sum-of-squares (ACT
     Square with accum) per L-quarter; group aggregation and broadcast-back
     via tiny matmuls with indicator matrices; xn = x*s - t' in place
     (written as float32r).
  2. Streamed QKV+logits: per 128-wide L-tile, q/k tiles (L on partitions)
     via matmul, bias added during PSUM evacuation, then 4 logits matmuls
     accumulate into a persistent 4-bank PSUM tile (512x512 logits).
  3. Softmax: negated row-max (DVE), Exp with fused row-sum (ACT), recip.
     Attention is kept unnormalized; the 1/rowsum is folded into the ret
     evacuation as a per-partition scale.
  4. attn^T via 16 PE transposes.
  5. Per 512-wide L-chunk: v = Wv@xn (+bias at evac), ret = attnT.T@v
     (x recip at evac), hproj = projT.T@ret; residual + proj bias fused
     into one scalar_tensor_tensor evacuation; DMA out.

All large matmuls run in float32r (1 cycle/row on the PE at N=512,
~11-bit-mantissa operand rounding, fp32 accumulation).
"""
import sys

if "/opt/trn_rl_repo" not in sys.path:
    sys.path.insert(0, "/opt/trn_rl_repo")

import math
import numpy as np

B = 32
C = 512
H = 64
Wd = 64
L = H * Wd            # 4096
NCORES = 8
SPC = B // NCORES     # samples per core = 4
CT = C // 128         # c-tiles = 4
LT = L // 128         # l-tiles in phase B = 32
NQ = 4                # L-quarters for x streaming
LQ = L // NQ          # 1024
NCH = 8               # output L-chunks
LCH = L // NCH        # 512
GROUPS = 32
GSIZE = C // GROUPS   # 16 channels per group
GPT = 128 // GSIZE    # groups per c-tile = 8
EPS = 1e-5
SCALE = 1.0 / math.sqrt(L)

_prog = None


def _round_fp32r(x):
    """Round fp32 to float32r (11 explicit mantissa bits), RNE."""
    u = np.ascontiguousarray(x, dtype=np.float32).view(np.uint32).astype(np.uint64)
    u = u + 0x7FF + ((u >> 12) & 1)
    return (u & np.uint64(0xFFFFF000)).astype(np.uint32).view(np.float32)


def _build_program():
    import concourse.bacc as bacc
    import concourse.bass as bass
    import concourse.tile as tile
    from concourse import mybir

    f32 = mybir.dt.float32
    f32r = mybir.dt.float32r
    Alu = mybir.AluOpType
    Act = mybir.ActivationFunctionType

    nc = bacc.Bacc("TRN2", target_bir_lowering=False, debug=False,
                   num_devices=NCORES)

    x_d = nc.declare_dram_parameter("x", [SPC, C, L], f32r, isOutput=False)
    wqk_d = nc.declare_dram_parameter("wqkT", [C, 3 * C], f32r, isOutput=False)
    pjT_d = nc.declare_dram_parameter("projT", [C, C], f32r, isOutput=False)
    bqk_d = nc.declare_dram_parameter("bqk", [2, C], f32, isOutput=False)
    bv_d = nc.declare_dram_parameter("bv_pk", [128, CT], f32, isOutput=False)
    pb_d = nc.declare_dram_parameter("pb_pk", [128, CT], f32, isOutput=False)
    gw_d = nc.declare_dram_parameter("gw_pk", [128, CT], f32, isOutput=False)
    gb_d = nc.declare_dram_parameter("gb_pk", [128, CT], f32, isOutput=False)
    g_d = nc.declare_dram_parameter("G", [128, GPT], f32, isOutput=False)
    b8_d = nc.declare_dram_parameter("B8", [GPT, 128], f32, isOutput=False)
    eye_d = nc.declare_dram_parameter("eye", [128, 128], f32r, isOutput=False)
    out_d = nc.declare_dram_parameter("out", [SPC, C, L], f32, isOutput=True)

    with tile.TileContext(nc) as tc:
        with tc.tile_pool(name="const", bufs=1) as cst, \
             tc.tile_pool(name="xq", bufs=5) as xqp, \
             tc.tile_pool(name="qk", bufs=3) as qkp, \
             tc.tile_pool(name="attn", bufs=1) as atp, \
             tc.tile_pool(name="chunk", bufs=2) as chp, \
             tc.tile_pool(name="stat", bufs=2) as stp, \
             tc.tile_pool(name="ps", bufs=1, space="PSUM") as psp:

            # ---- constants (loaded once) ----
            wqk_sb = cst.tile([128, CT, 3 * C], f32r, tag="wqk")
            nc.sync.dma_start(out=wqk_sb,
                              in_=wqk_d.ap().rearrange("(t p) o -> p t o", p=128))
            pjT_sb = cst.tile([128, CT, C], f32r, tag="pjT")
            nc.sync.dma_start(out=pjT_sb,
                              in_=pjT_d.ap().rearrange("(t p) o -> p t o", p=128))
            bq_bc = cst.tile([128, C], f32, tag="bq")
            nc.sync.dma_start(out=bq_bc, in_=bass.AP(
                tensor=bqk_d, offset=0, ap=[[0, 128], [1, C]]))
            bk_bc = cst.tile([128, C], f32, tag="bk")
            nc.sync.dma_start(out=bk_bc, in_=bass.AP(
                tensor=bqk_d, offset=C, ap=[[0, 128], [1, C]]))
            bv_sb = cst.tile([128, CT], f32, tag="bv")
            nc.sync.dma_start(out=bv_sb, in_=bv_d.ap())
            pb_sb = cst.tile([128, CT], f32, tag="pb")
            nc.sync.dma_start(out=pb_sb, in_=pb_d.ap())
            gw_sb = cst.tile([128, CT], f32, tag="gw")
            nc.sync.dma_start(out=gw_sb, in_=gw_d.ap())
            gb_sb = cst.tile([128, CT], f32, tag="gb")
            nc.sync.dma_start(out=gb_sb, in_=gb_d.ap())
            g_sb = cst.tile([128, GPT], f32, tag="G")
            nc.sync.dma_start(out=g_sb, in_=g_d.ap())
            b8_sb = cst.tile([GPT, 128], f32, tag="B8")
            nc.sync.dma_start(out=b8_sb, in_=b8_d.ap())
            eye_sb = cst.tile([128, 128], f32r, tag="eye")
            nc.sync.dma_start(out=eye_sb, in_=eye_d.ap())
            eps_sb = cst.tile([GPT, 1], f32, tag="eps")
            nc.vector.memset(eps_sb, EPS)

            for s in range(SPC):
                x_s = x_d.ap()[s].rearrange("(t p) l -> p t l", p=128)
                o_s = out_d.ap()[s].rearrange("(t p) l -> p t l", p=128)

                # ---- load x quarters + per-partition stats ----
                xq = []
                st4 = stp.tile([128, CT, 2, NQ], f32, tag="st4")
                for iq in range(NQ):
                    xt = xqp.tile([128, CT, LQ], f32r, tag="xq")
                    nc.sync.dma_start(out=xt, in_=x_s[:, :, iq * LQ:(iq + 1) * LQ])
                    xq.append(xt)
                    for ct in range(CT):
                        nc.vector.tensor_reduce(
                            out=st4[:, ct, 0, iq:iq + 1],
                            in_=xt[:, ct, :].bitcast(f32),
                            axis=mybir.AxisListType.X, op=Alu.add)
                        scr = stp.tile([128, LQ], f32, tag="scr")
                        nc.scalar.activation(
                            out=scr, in_=xt[:, ct, :].bitcast(f32),
                            func=Act.Square,
                            accum_out=st4[:, ct, 1, iq:iq + 1])

                # combine quarters -> [sum, sumsq] per partition
                st2 = stp.tile([128, CT, 2], f32, tag="st2")
                nc.vector.tensor_reduce(out=st2, in_=st4,
                                        axis=mybir.AxisListType.X, op=Alu.add)

                # group aggregation: gs[:, ct, :] = [mean_g, E[x^2]_g] (8 groups)
                gs = stp.tile([GPT, CT, 2], f32, tag="gs")
                for ct in range(CT):
                    gps = psp.tile([GPT, 2], f32, tag="mm", bufs=4)
                    nc.tensor.matmul(gps, g_sb[:, :], st2[:, ct, :],
                                     start=True, stop=True)
                    nc.vector.tensor_copy(out=gs[:, ct, :], in_=gps)
                # var = E[x^2] - mean^2 ; rstd = 1/sqrt(var+eps)  (in place)
                msq = stp.tile([GPT, CT], f32, tag="msq")
                nc.vector.tensor_tensor(out=msq, in0=gs[:, :, 0], in1=gs[:, :, 0],
                                        op=Alu.mult)
                nc.vector.tensor_tensor(out=gs[:, :, 1], in0=gs[:, :, 1], in1=msq,
                                        op=Alu.subtract)
                nc.scalar.activation(out=gs[:, :, 1], in_=gs[:, :, 1],
                                     func=Act.Sqrt, bias=eps_sb[:, :])
                nc.vector.reciprocal(out=gs[:, :, 1], in_=gs[:, :, 1])

                # broadcast back to channels; s_c = rstd*gn_w, t'_c = mean*s - gn_b
                s_sb = stp.tile([128, CT], f32, tag="s_sb")
                t_sb = stp.tile([128, CT], f32, tag="t_sb")
                for ct in range(CT):
                    bps = psp.tile([128, 2], f32, tag="mm", bufs=4)
                    nc.tensor.matmul(bps, b8_sb[:, :], gs[:, ct, :],
                                     start=True, stop=True)
                    nc.vector.tensor_tensor(out=s_sb[:, ct:ct + 1],
                                            in0=bps[:, 1:2],
                                            in1=gw_sb[:, ct:ct + 1], op=Alu.mult)
                    nc.vector.scalar_tensor_tensor(
                        out=t_sb[:, ct:ct + 1], in0=bps[:, 0:1],
                        scalar=s_sb[:, ct:ct + 1], in1=gb_sb[:, ct:ct + 1],
                        op0=Alu.mult, op1=Alu.subtract)

                # normalize in place: xn = x*s - t'  (written as f32r)
                for iq in range(NQ):
                    for ct in range(CT):
                        nc.vector.tensor_scalar(
                            out=xq[iq][:, ct, :],
                            in0=xq[iq][:, ct, :].bitcast(f32),
                            scalar1=s_sb[:, ct:ct + 1],
                            scalar2=t_sb[:, ct:ct + 1],
                            op0=Alu.mult, op1=Alu.subtract)

                def xn(ct, lo, ln):
                    return xq[lo // LQ][:, ct, lo % LQ:lo % LQ + ln]

                # ---- phase B: q/k stream + logits accumulation ----
                lg = psp.tile([128, CT, C], f32, tag="lg", bufs=1)
                for il in range(LT):
                    lo = il * 128
                    qps = psp.tile([128, C], f32, tag="mm", bufs=4)
                    for ct in range(CT):
                        nc.tensor.matmul(qps, xn(ct, lo, 128),
                                         wqk_sb[:, ct, 0:C],
                                         start=(ct == 0), stop=(ct == CT - 1))
                    kps = psp.tile([128, C], f32, tag="mm", bufs=4)
                    for ct in range(CT):
                        nc.tensor.matmul(kps, xn(ct, lo, 128),
                                         wqk_sb[:, ct, C:2 * C],
                                         start=(ct == 0), stop=(ct == CT - 1))
                    q_sb = qkp.tile([128, C], f32r, tag="q")
                    nc.vector.tensor_tensor(out=q_sb, in0=qps, in1=bq_bc,
                                            op=Alu.add)
                    k_sb = qkp.tile([128, C], f32r, tag="k")
                    nc.vector.tensor_tensor(out=k_sb, in0=kps, in1=bk_bc,
                                            op=Alu.add)
                    for j in range(CT):
                        nc.tensor.matmul(lg[:, j, :],
                                         q_sb[:, 128 * j:128 * (j + 1)], k_sb,
                                         start=(il == 0), stop=(il == LT - 1))

                # ---- softmax (unnormalized) ----
                nmx = stp.tile([128, CT], f32, tag="nmx")
                rsum = stp.tile([128, CT], f32, tag="rsum")
                rcp = stp.tile([128, CT], f32, tag="rcp")
                p_sb = atp.tile([128, CT, C], f32r, tag="p_sb")
                for j in range(CT):
                    nc.vector.tensor_reduce(out=nmx[:, j:j + 1], in_=lg[:, j, :],
                                            axis=mybir.AxisListType.X,
                                            op=Alu.max, negate=True)
                    nc.scalar.activation(out=p_sb[:, j, :], in_=lg[:, j, :],
                                         func=Act.Exp, bias=nmx[:, j:j + 1],
                                         accum_out=rsum[:, j:j + 1])
                    nc.vector.reciprocal(out=rcp[:, j:j + 1], in_=rsum[:, j:j + 1])

                # ---- transpose attn: attnT[k, q] ----
                at_sb = atp.tile([128, CT, C], f32r, tag="at_sb")
                for t in range(CT):
                    for j in range(CT):
                        tps = psp.tile([128, 128], f32r, tag="mm", bufs=4)
                        nc.tensor.transpose(tps,
                                            p_sb[:, j, 128 * t:128 * (t + 1)],
                                            eye_sb[:, :])
                        nc.scalar.copy(out=at_sb[:, t, 128 * j:128 * (j + 1)],
                                       in_=tps)

                # ---- chunks: v -> ret -> out ----
                for ic in range(NCH):
                    lo = ic * LCH
                    v_sb = chp.tile([128, CT, LCH], f32r, tag="v_sb")
                    for m in range(CT):
                        vps = psp.tile([128, LCH], f32, tag="mm", bufs=4)
                        for ct in range(CT):
                            nc.tensor.matmul(
                                vps, wqk_sb[:, ct, 2 * C + 128 * m:
                                            2 * C + 128 * (m + 1)],
                                xn(ct, lo, LCH),
                                start=(ct == 0), stop=(ct == CT - 1))
                        nc.scalar.activation(out=v_sb[:, m, :], in_=vps,
                                             func=Act.Identity,
                                             bias=bv_sb[:, m:m + 1])
                    ret_sb = chp.tile([128, CT, LCH], f32r, tag="ret_sb")
                    for j in range(CT):
                        rps = psp.tile([128, LCH], f32, tag="mm", bufs=4)
                        for t in range(CT):
                            nc.tensor.matmul(rps,
                                             at_sb[:, t, 128 * j:128 * (j + 1)],
                                             v_sb[:, t, :],
                                             start=(t == 0), stop=(t == CT - 1))
                        nc.scalar.activation(out=ret_sb[:, j, :], in_=rps,
                                             func=Act.Identity,
                                             scale=rcp[:, j:j + 1])
                    o_sb = chp.tile([128, CT, LCH], f32, tag="o_sb")
                    for m in range(CT):
                        ops = psp.tile([128, LCH], f32, tag="mm", bufs=4)
                        for ct in range(CT):
                            nc.tensor.matmul(ops,
                                             pjT_sb[:, ct, 128 * m:128 * (m + 1)],
                                             ret_sb[:, ct, :],
                                             start=(ct == 0), stop=(ct == CT - 1))
                        nc.vector.scalar_tensor_tensor(
                            out=o_sb[:, m, :], in0=ops,
                            scalar=pb_sb[:, m:m + 1],
                            in1=xn(m, lo, LCH).bitcast(f32),
                            op0=Alu.add, op1=Alu.add)
                    nc.sync.dma_start(out=o_s[:, :, lo:lo + LCH], in_=o_sb)

    nc.compile()
    return nc


def _get_program():
    global _prog
    if _prog is None:
        _prog = _build_program()
    return _prog


def kernel(x, gn_w, gn_b, qkv_w, qkv_b, proj_w, proj_b):
    from concourse.bass_utils import run_bass_kernel_spmd

    nc = _get_program()

    x = _round_fp32r(np.asarray(x, dtype=np.float32)).reshape(B, C, L)
    qkv_w = np.asarray(qkv_w, dtype=np.float32)
    qkv_b = np.asarray(qkv_b, dtype=np.float32)
    proj_w = np.asarray(proj_w, dtype=np.float32)
    proj_b = np.asarray(proj_b, dtype=np.float32)
    gn_w = np.asarray(gn_w, dtype=np.float32)
    gn_b = np.asarray(gn_b, dtype=np.float32)

    wqkT = np.ascontiguousarray(qkv_w.T).copy()      # (C, 3C)
    wqkT[:, :C] *= SCALE
    wqkT = _round_fp32r(wqkT)
    projT = _round_fp32r(np.ascontiguousarray(proj_w.T))
    bqk = np.stack([qkv_b[:C] * SCALE, qkv_b[C:2 * C]]).astype(np.float32)
    bv_pk = np.ascontiguousarray(qkv_b[2 * C:].reshape(CT, 128).T)
    pb_pk = np.ascontiguousarray(proj_b.reshape(CT, 128).T)
    gw_pk = np.ascontiguousarray(gn_w.reshape(CT, 128).T)
    gb_pk = np.ascontiguousarray(gn_b.reshape(CT, 128).T)
    G = ((np.arange(128)[:, None] // GSIZE == np.arange(GPT)[None, :])
         .astype(np.float32) / (GSIZE * L))
    B8 = (np.arange(GPT)[:, None] == np.arange(128)[None, :] // GSIZE
          ).astype(np.float32)
    eye = np.eye(128, dtype=np.float32)

    shared = {"wqkT": wqkT, "projT": projT, "bqk": bqk, "bv_pk": bv_pk,
              "pb_pk": pb_pk, "gw_pk": gw_pk, "gb_pk": gb_pk,
              "G": G, "B8": B8, "eye": eye}
    in_maps = [{"x": x[c * SPC:(c + 1) * SPC], **shared} for c in range(NCORES)]

    res = run_bass_kernel_spmd(nc, in_maps, list(range(NCORES)))
    out = np.concatenate([r["out"] for r in res.results], axis=0)
    return out.reshape(B, C, H, Wd)


# revision 6
# speedup vs baseline: 382.2133x; 382.2133x over previous
"""Trainium2 Bass kernel for nn_AttentionBlock (B=32, C=512, H=W=64, GROUPS=32).

Sharding: data-parallel over batch — 4 samples per core on 8 NeuronCores,
no collectives. Host preprocesses weights (transpose, prescale q by
1/sqrt(L), round to float32r) and sends small indicator matrices used for
cross-partition GroupNorm reductions on the tensor engine.

Per-sample dataflow on one core (x viewed as (C=512, L=4096), channels on
partitions, 4 c-tiles of 128):
  1. GroupNorm stats: per-partition sum (DVE reduce) + sum-of-squares (ACT
     Square with accum) per L-quarter; group aggregation and broadcast-back
     via tiny matmuls with indicator matrices; xn = x*s - t' in place
     (written as float32r).
  2. Streamed QKV+logits: per 128-wide L-tile, q/k tiles (L on partitions)
     via matmul, bias added during PSUM evacuation, then 4 logits matmuls
     accumulate into a persistent 4-bank PSUM tile (512x512 logits).
  3. Softmax: negated row-max (DVE), Exp with fused row-sum (ACT), recip.
     Attention is kept unnormalized; the 1/rowsum is folded into the ret
     evacuation as a per-partition scale.
  4. attn^T via 16 PE transposes.
  5. Per 512-wide L-chunk: v = Wv@xn (+bias at evac), ret = attnT.T@v
     (x recip at evac), hproj = projT.T@ret; residual + proj bias fused
     into one scalar_tensor_tensor evacuation; DMA out.

All large matmuls run in float32r (1 cycle/row on the PE at N=512,
~11-bit-mantissa operand rounding, fp32 accumulation).
"""
import sys

if "/opt/trn_rl_repo" not in sys.path:
    sys.path.insert(0, "/opt/trn_rl_repo")

import math
import numpy as np

B = 32
C = 512
H = 64
Wd = 64
L = H * Wd            # 4096
NCORES = 8
SPC = B // NCORES     # samples per core = 4
CT = C // 128         # c-tiles = 4
LT = L // 128         # l-tiles in phase B = 32
NQ = 4                # L-quarters for x streaming
LQ = L // NQ          # 1024
NCH = 8               # output L-chunks
LCH = L // NCH        # 512
GROUPS = 32
GSIZE = C // GROUPS   # 16 channels per group
GPT = 128 // GSIZE    # groups per c-tile = 8
EPS = 1e-5
SCALE = 1.0 / math.sqrt(L)

_prog = None


def _round_fp32r(x):
    """Round fp32 to float32r (11 explicit mantissa bits), RNE."""
    u = np.ascontiguousarray(x, dtype=np.float32).view(np.uint32).astype(np.uint64)
    u = u + 0x7FF + ((u >> 12) & 1)
    return (u & np.uint64(0xFFFFF000)).astype(np.uint32).view(np.float32)


def _build_program(spc=SPC):
    import concourse.bacc as bacc
    import concourse.bass as bass
    import concourse.tile as tile
    from concourse import mybir

    f32 = mybir.dt.float32
    f32r = mybir.dt.float32r
    Alu = mybir.AluOpType
    Act = mybir.ActivationFunctionType

    nc = bacc.Bacc("TRN2", target_bir_lowering=False, debug=False,
                   num_devices=NCORES)

    x_d = nc.declare_dram_parameter("x", [spc, C, L], f32r, isOutput=False)
    wqk_d = nc.declare_dram_parameter("wqkT", [C, 3 * C], f32r, isOutput=False)
    pjT_d = nc.declare_dram_parameter("projT", [C, C], f32r, isOutput=False)
    bqk_d = nc.declare_dram_parameter("bqk", [2, C], f32, isOutput=False)
    bv_d = nc.declare_dram_parameter("bv_pk", [128, CT], f32, isOutput=False)
    pb_d = nc.declare_dram_parameter("pb_pk", [128, CT], f32, isOutput=False)
    gw_d = nc.declare_dram_parameter("gw_pk", [128, CT], f32, isOutput=False)
    gb_d = nc.declare_dram_parameter("gb_pk", [128, CT], f32, isOutput=False)
    g_d = nc.declare_dram_parameter("G", [128, GPT], f32, isOutput=False)
    b8_d = nc.declare_dram_parameter("B8", [GPT, 128], f32, isOutput=False)
    eye_d = nc.declare_dram_parameter("eye", [128, 128], f32r, isOutput=False)
    out_d = nc.declare_dram_parameter("out", [spc, C, L], f32, isOutput=True)

    with tile.TileContext(nc) as tc:
        with tc.tile_pool(name="const", bufs=1) as cst, \
             tc.tile_pool(name="xq", bufs=5) as xqp, \
             tc.tile_pool(name="qk", bufs=3) as qkp, \
             tc.tile_pool(name="attn", bufs=1) as atp, \
             tc.tile_pool(name="chunk", bufs=2) as chp, \
             tc.tile_pool(name="stat", bufs=2) as stp, \
             tc.tile_pool(name="ps", bufs=1, space="PSUM") as psp:

            # ---- constants (loaded once) ----
            wqk_sb = cst.tile([128, CT, 3 * C], f32r, tag="wqk")
            nc.sync.dma_start(out=wqk_sb,
                              in_=wqk_d.ap().rearrange("(t p) o -> p t o", p=128))
            pjT_sb = cst.tile([128, CT, C], f32r, tag="pjT")
            nc.sync.dma_start(out=pjT_sb,
                              in_=pjT_d.ap().rearrange("(t p) o -> p t o", p=128))
            bq_bc = cst.tile([128, C], f32, tag="bq")
            nc.sync.dma_start(out=bq_bc, in_=bass.AP(
                tensor=bqk_d, offset=0, ap=[[0, 128], [1, C]]))
            bk_bc = cst.tile([128, C], f32, tag="bk")
            nc.sync.dma_start(out=bk_bc, in_=bass.AP(
                tensor=bqk_d, offset=C, ap=[[0, 128], [1, C]]))
            bv_sb = cst.tile([128, CT], f32, tag="bv")
            nc.sync.dma_start(out=bv_sb, in_=bv_d.ap())
            pb_sb = cst.tile([128, CT], f32, tag="pb")
            nc.sync.dma_start(out=pb_sb, in_=pb_d.ap())
            gw_sb = cst.tile([128, CT], f32, tag="gw")
            nc.sync.dma_start(out=gw_sb, in_=gw_d.ap())
            gb_sb = cst.tile([128, CT], f32, tag="gb")
            nc.sync.dma_start(out=gb_sb, in_=gb_d.ap())
            g_sb = cst.tile([128, GPT], f32, tag="G")
            nc.sync.dma_start(out=g_sb, in_=g_d.ap())
            b8_sb = cst.tile([GPT, 128], f32, tag="B8")
            nc.sync.dma_start(out=b8_sb, in_=b8_d.ap())
            eye_sb = cst.tile([128, 128], f32r, tag="eye")
            nc.sync.dma_start(out=eye_sb, in_=eye_d.ap())
            eps_sb = cst.tile([GPT, 1], f32, tag="eps")
            nc.vector.memset(eps_sb, EPS)

            for s in range(spc):
                x_s = x_d.ap()[s].rearrange("(t p) l -> p t l", p=128)
                o_s = out_d.ap()[s].rearrange("(t p) l -> p t l", p=128)

                # ---- load x quarters + per-partition stats ----
                xq = []
                st4 = stp.tile([128, CT, 2, NQ], f32, tag="st4")
                for iq in range(NQ):
                    xt = xqp.tile([128, CT, LQ], f32r, tag="xq")
                    nc.sync.dma_start(out=xt, in_=x_s[:, :, iq * LQ:(iq + 1) * LQ])
                    xq.append(xt)
                    for ct in range(CT):
                        nc.vector.tensor_reduce(
                            out=st4[:, ct, 0, iq:iq + 1],
                            in_=xt[:, ct, :].bitcast(f32),
                            axis=mybir.AxisListType.X, op=Alu.add)
                        scr = stp.tile([128, LQ], f32, tag="scr")
                        nc.scalar.activation(
                            out=scr, in_=xt[:, ct, :].bitcast(f32),
                            func=Act.Square,
                            accum_out=st4[:, ct, 1, iq:iq + 1])

                # combine quarters -> [sum, sumsq] per partition
                st2 = stp.tile([128, CT, 2], f32, tag="st2")
                nc.vector.tensor_reduce(out=st2, in_=st4,
                                        axis=mybir.AxisListType.X, op=Alu.add)

                # group aggregation: gs[:, ct, :] = [mean_g, E[x^2]_g] (8 groups)
                gs = stp.tile([GPT, CT, 2], f32, tag="gs")
                for ct in range(CT):
                    gps = psp.tile([GPT, 2], f32, tag="mm", bufs=4)
                    nc.tensor.matmul(gps, g_sb[:, :], st2[:, ct, :],
                                     start=True, stop=True)
                    nc.vector.tensor_copy(out=gs[:, ct, :], in_=gps)
                # var = E[x^2] - mean^2 ; rstd = 1/sqrt(var+eps)  (in place)
                msq = stp.tile([GPT, CT], f32, tag="msq")
                nc.vector.tensor_tensor(out=msq, in0=gs[:, :, 0], in1=gs[:, :, 0],
                                        op=Alu.mult)
                nc.vector.tensor_tensor(out=gs[:, :, 1], in0=gs[:, :, 1], in1=msq,
                                        op=Alu.subtract)
                nc.scalar.activation(out=gs[:, :, 1], in_=gs[:, :, 1],
                                     func=Act.Sqrt, bias=eps_sb[:, :])
                nc.vector.reciprocal(out=gs[:, :, 1], in_=gs[:, :, 1])

                # broadcast back to channels; s_c = rstd*gn_w, t'_c = mean*s - gn_b
                s_sb = stp.tile([128, CT], f32, tag="s_sb")
                t_sb = stp.tile([128, CT], f32, tag="t_sb")
                for ct in range(CT):
                    bps = psp.tile([128, 2], f32, tag="mm", bufs=4)
                    nc.tensor.matmul(bps, b8_sb[:, :], gs[:, ct, :],
                                     start=True, stop=True)
                    nc.vector.tensor_tensor(out=s_sb[:, ct:ct + 1],
                                            in0=bps[:, 1:2],
                                            in1=gw_sb[:, ct:ct + 1], op=Alu.mult)
                    nc.vector.scalar_tensor_tensor(
                        out=t_sb[:, ct:ct + 1], in0=bps[:, 0:1],
                        scalar=s_sb[:, ct:ct + 1], in1=gb_sb[:, ct:ct + 1],
                        op0=Alu.mult, op1=Alu.subtract)

                # normalize in place: xn = x*s - t'  (written as f32r)
                for iq in range(NQ):
                    for ct in range(CT):
                        nc.vector.tensor_scalar(
                            out=xq[iq][:, ct, :],
                            in0=xq[iq][:, ct, :].bitcast(f32),
                            scalar1=s_sb[:, ct:ct + 1],
                            scalar2=t_sb[:, ct:ct + 1],
                            op0=Alu.mult, op1=Alu.subtract)

                def xn(ct, lo, ln):
                    return xq[lo // LQ][:, ct, lo % LQ:lo % LQ + ln]

                # ---- phase B: q/k stream + logits accumulation ----
                lg = psp.tile([128, CT, C], f32, tag="lg", bufs=1)
                for il in range(LT):
                    lo = il * 128
                    qps = psp.tile([128, C], f32, tag="mm", bufs=4)
                    for ct in range(CT):
                        nc.tensor.matmul(qps, xn(ct, lo, 128),
                                         wqk_sb[:, ct, 0:C],
                                         start=(ct == 0), stop=(ct == CT - 1))
                    kps = psp.tile([128, C], f32, tag="mm", bufs=4)
                    for ct in range(CT):
                        nc.tensor.matmul(kps, xn(ct, lo, 128),
                                         wqk_sb[:, ct, C:2 * C],
                                         start=(ct == 0), stop=(ct == CT - 1))
                    q_sb = qkp.tile([128, C], f32r, tag="q")
                    nc.vector.tensor_tensor(out=q_sb, in0=qps, in1=bq_bc,
                                            op=Alu.add)
                    k_sb = qkp.tile([128, C], f32r, tag="k")
                    nc.vector.tensor_tensor(out=k_sb, in0=kps, in1=bk_bc,
                                            op=Alu.add)
                    for j in range(CT):
                        nc.tensor.matmul(lg[:, j, :],
                                         q_sb[:, 128 * j:128 * (j + 1)], k_sb,
                                         start=(il == 0), stop=(il == LT - 1))

                # ---- softmax (unnormalized) ----
                nmx = stp.tile([128, CT], f32, tag="nmx")
                rsum = stp.tile([128, CT], f32, tag="rsum")
                rcp = stp.tile([128, CT], f32, tag="rcp")
                p_sb = atp.tile([128, CT, C], f32r, tag="p_sb")
                for j in range(CT):
                    nc.vector.tensor_reduce(out=nmx[:, j:j + 1], in_=lg[:, j, :],
                                            axis=mybir.AxisListType.X,
                                            op=Alu.max, negate=True)
                    nc.scalar.activation(out=p_sb[:, j, :], in_=lg[:, j, :],
                                         func=Act.Exp, bias=nmx[:, j:j + 1],
                                         accum_out=rsum[:, j:j + 1])
                    nc.vector.reciprocal(out=rcp[:, j:j + 1], in_=rsum[:, j:j + 1])

                # ---- transpose attn: attnT[k, q] ----
                at_sb = atp.tile([128, CT, C], f32r, tag="at_sb")
                for t in range(CT):
                    for j in range(CT):
                        tps = psp.tile([128, 128], f32r, tag="mm", bufs=4)
                        nc.tensor.transpose(tps,
                                            p_sb[:, j, 128 * t:128 * (t + 1)],
                                            eye_sb[:, :])
                        nc.scalar.copy(out=at_sb[:, t, 128 * j:128 * (j + 1)],
                                       in_=tps)

                # ---- chunks: v -> ret -> out ----
                for ic in range(NCH):
                    lo = ic * LCH
                    v_sb = chp.tile([128, CT, LCH], f32r, tag="v_sb")
                    for m in range(CT):
                        vps = psp.tile([128, LCH], f32, tag="mm", bufs=4)
                        for ct in range(CT):
                            nc.tensor.matmul(
                                vps, wqk_sb[:, ct, 2 * C + 128 * m:
                                            2 * C + 128 * (m + 1)],
                                xn(ct, lo, LCH),
                                start=(ct == 0), stop=(ct == CT - 1))
                        nc.scalar.activation(out=v_sb[:, m, :], in_=vps,
                                             func=Act.Identity,
                                             bias=bv_sb[:, m:m + 1])
                    ret_sb = chp.tile([128, CT, LCH], f32r, tag="ret_sb")
                    for j in range(CT):
                        rps = psp.tile([128, LCH], f32, tag="mm", bufs=4)
                        for t in range(CT):
                            nc.tensor.matmul(rps,
                                             at_sb[:, t, 128 * j:128 * (j + 1)],
                                             v_sb[:, t, :],
                                             start=(t == 0), stop=(t == CT - 1))
                        nc.scalar.activation(out=ret_sb[:, j, :], in_=rps,
                                             func=Act.Identity,
                                             scale=rcp[:, j:j + 1])
                    o_sb = chp.tile([128, CT, LCH], f32, tag="o_sb")
                    for m in range(CT):
                        ops = psp.tile([128, LCH], f32, tag="mm", bufs=4)
                        for ct in range(CT):
                            nc.tensor.matmul(ops,
                                             pjT_sb[:, ct, 128 * m:128 * (m + 1)],
                                             ret_sb[:, ct, :],
                                             start=(ct == 0), stop=(ct == CT - 1))
                        nc.vector.scalar_tensor_tensor(
                            out=o_sb[:, m, :], in0=ops,
                            scalar=pb_sb[:, m:m + 1],
                            in1=xn(m, lo, LCH).bitcast(f32),
                            op0=Alu.add, op1=Alu.add)
                    nc.sync.dma_start(out=o_s[:, :, lo:lo + LCH], in_=o_sb)

    nc.compile()
    return nc


def _get_program():
    global _prog
    if _prog is None:
        _prog = _build_program()
    return _prog


def kernel(x, gn_w, gn_b, qkv_w, qkv_b, proj_w, proj_b):
    from concourse.bass_utils import run_bass_kernel_spmd

    nc = _get_program()

    x = _round_fp32r(np.asarray(x, dtype=np.float32)).reshape(B, C, L)
    qkv_w = np.asarray(qkv_w, dtype=np.float32)
    qkv_b = np.asarray(qkv_b, dtype=np.float32)
    proj_w = np.asarray(proj_w, dtype=np.float32)
    proj_b = np.asarray(proj_b, dtype=np.float32)
    gn_w = np.asarray(gn_w, dtype=np.float32)
    gn_b = np.asarray(gn_b, dtype=np.float32)

    wqkT = np.ascontiguousarray(qkv_w.T).copy()      # (C, 3C)
    wqkT[:, :C] *= SCALE
    wqkT = _round_fp32r(wqkT)
    projT = _round_fp32r(np.ascontiguousarray(proj_w.T))
    bqk = np.stack([qkv_b[:C] * SCALE, qkv_b[C:2 * C]]).astype(np.float32)
    bv_pk = np.ascontiguousarray(qkv_b[2 * C:].reshape(CT, 128).T)
    pb_pk = np.ascontiguousarray(proj_b.reshape(CT, 128).T)
    gw_pk = np.ascontiguousarray(gn_w.reshape(CT, 128).T)
    gb_pk = np.ascontiguousarray(gn_b.reshape(CT, 128).T)
    G = ((np.arange(128)[:, None] // GSIZE == np.arange(GPT)[None, :])
         .astype(np.float32) / (GSIZE * L))
    B8 = (np.arange(GPT)[:, None] == np.arange(128)[None, :] // GSIZE
          ).astype(np.float32)
    eye = np.eye(128, dtype=np.float32)

    shared = {"wqkT": wqkT, "projT": projT, "bqk": bqk, "bv_pk": bv_pk,
              "pb_pk": pb_pk, "gw_pk": gw_pk, "gb_pk": gb_pk,
              "G": G, "B8": B8, "eye": eye}
    in_maps = [{"x": x[c * SPC:(c + 1) * SPC], **shared} for c in range(NCORES)]

    res = run_bass_kernel_spmd(nc, in_maps, list(range(NCORES)))
    out = np.concatenate([r["out"] for r in res.results], axis=0)
    return out.reshape(B, C, H, Wd)
